# revision 42
# baseline (speedup 1.0000x reference)
"""Trainium2 Bass kernel for nn_DenseExpert (soft-gated mixture of dense experts).

Math:  out[b,u] = sum_e gate[b,e] * (x[b,:] @ alpha[e]) [u] + (gate @ beta)[b,u]

Strategy (pure data parallel over batch, 8 cores). Host pre-casts everything to
bf16 and pre-arranges layouts (x chunked partition-major, gate both batch-major
and pre-transposed, alpha as [i,e,u]) so the device does no casts and no gate
transposes. Per core (8192 rows, 16 chunks of 512):
  1. One-shot DMAs land gate/gateT/alpha/beta/idrep in SBUF; x streams per
     chunk. A few warmup matmuls flip the PE HAM clock-gate to full rate
     while the first DMAs land.
  2. Per chunk, GpSimd builds the block-diagonal gate tensor
     diag[p, (t,e), c] = gate[p,c,(t,e)] * [c == p%64] (one tensor_tensor
     against a 0/1 pattern with a broadcast AP).
  3. Stage 1 (PE): per 128-row tile, 2 row-tiled concurrent matmuls
     (tile_position=(64l,0), K=64, N=512) compute the gate-scaled transpose
     yT[i, (e, c)] = x_block.T @ diag_block into single-bank fp32 PSUM tiles.
  4. DVE (l=0) and ACT (l=1) evacuate the two banks of each tile in parallel
     to bf16 [i, e, b] SBUF layout (1-bank tiles make PSUM slots recycle at
     half-tile granularity, which keeps the PE fed).
  5. Stage 2 (PE): out.T[u, b] = sum_e alpha_e.T @ yT_e accumulated in PSUM;
     the bias rides in the same accumulation group as a leading K=8 matmul
     beta.T @ gateT, and each expert is split into two N=256 halves so the
     first half only waits on the first two tiles' evacuations.
  6. out.T evacuated fp32->bf16 (DVE/ACT alternating per chunk), DMA'd in
     4-chunk groups; host concatenates, casts to fp32, transposes.
"""

import dataclasses
from contextlib import ExitStack

import numpy as np
import ml_dtypes

BF16 = ml_dtypes.bfloat16

import concourse.bacc as bacc
import concourse.tile as tile
import concourse.mybir as mybir
from concourse.bass_utils import run_bass_kernel_spmd

F32 = mybir.dt.float32
F16 = mybir.dt.bfloat16

B, E, I, U = 65536, 8, 128, 128
NCORES = 8
BLOC = B // NCORES        # 8192 batch rows per core
CHUNK = 512               # batch rows per pipeline chunk
NCHUNK = BLOC // CHUNK    # 16
TPC = CHUNK // 128        # 128-row tiles per chunk (4)
KB = 64                   # contraction block for the diag trick
NL = 128 // KB            # row-tile blocks per 128-row tile (4)
OGRP = 4                  # chunks per output DMA group


def _build():
    nc = bacc.Bacc("TRN2", target_bir_lowering=False, debug=False)

    # host-prearranged fp16 inputs
    xh = nc.dram_tensor("xh", [128, NCHUNK, TPC, I], F16, kind="ExternalInput").ap()
    gh = nc.dram_tensor("gh", [128, NCHUNK, TPC, E], F16, kind="ExternalInput").ap()
    gT = nc.dram_tensor("gT", [E, BLOC], F16, kind="ExternalInput").ap()
    alpha = nc.dram_tensor("alpha", [I, E, U], F16, kind="ExternalInput").ap()
    beta = nc.dram_tensor("beta", [E, U], F16, kind="ExternalInput").ap()
    idrep = nc.dram_tensor("idrep", [128, E, KB], F16, kind="ExternalInput").ap()
    # output stays feature-major fp16 on HW; host casts + transposes
    outT = nc.dram_tensor("outT", [U, BLOC], F16, kind="ExternalOutput").ap()

    with tile.TileContext(nc) as tc, ExitStack() as ctx:
        const = ctx.enter_context(tc.tile_pool(name="const", bufs=1))
        xp = ctx.enter_context(tc.tile_pool(name="xp", bufs=6))
        dgp = ctx.enter_context(tc.tile_pool(name="dgp", bufs=3))
        ytp = ctx.enter_context(tc.tile_pool(name="ytp", bufs=3))
        op = ctx.enter_context(tc.tile_pool(name="op", bufs=2))
        ps_yt = ctx.enter_context(tc.tile_pool(name="ps_yt", bufs=6, space="PSUM"))
        ps_ot = ctx.enter_context(tc.tile_pool(name="ps_ot", bufs=2, space="PSUM"))

        # --- one-shot input DMAs (x streamed per chunk below) ---
        idrep_sb = const.tile([128, E, KB], F16, tag="idrep")
        nc.sync.dma_start(idrep_sb[:], idrep)
        gh_sb = const.tile([128, NCHUNK, TPC, E], F16, tag="gh")
        nc.sync.dma_start(gh_sb[:], gh)
        beta_sb = const.tile([E, U], F16, tag="beta")
        nc.sync.dma_start(beta_sb[:], beta)
        alpha_sb = const.tile([I, E, U], F16, tag="alpha")
        nc.sync.dma_start(alpha_sb[:], alpha)
        gT_sb = const.tile([E, BLOC], F16, tag="gT")
        nc.sync.dma_start(gT_sb[:], gT)


        # --- PE warmup: flip HAM to 8/8 while input DMAs land; the same
        # scratch tile is reused for keep-alive matmuls during the ramp so
        # the clock gate never drops while GpSimd catches up on diag builds
        warm_ps = ps_yt.tile([KB, E * KB], F32, tag="yTps")

        def keep_alive(n):
            for _ in range(n):
                nc.tensor.matmul(
                    warm_ps[:],
                    idrep_sb[:, 0, :],
                    dataclasses.replace(
                        idrep_sb[:], ap=[[E * KB, 128], [1, E * KB]], offset=0
                    ),
                    start=True,
                    stop=True,
                )

        keep_alive(6)

        gh_flat = E * TPC * NCHUNK  # per-partition elements of gh_sb
        NSLOT = TPC * E
        DSLOT = 8

        def emit_diag(c, eng=nc.gpsimd):
            # diag[p,(te),cc] = gh[p,c,(te)]*[cc==p%KB], one op per chunk;
            # chunks 0-1 are built on DVE in the prologue (it is idle until
            # the first evacuations), so GpSimd starts directly on diag(2)
            diag = dgp.tile([128, TPC, E, KB], F16, tag="diag")
            dst = dataclasses.replace(
                diag[:],
                ap=[[NSLOT * KB, 128], [KB, NSLOT], [1, KB]],
                offset=0,
            )
            gv = dataclasses.replace(
                gh_sb[:],
                ap=[[gh_flat, 128], [1, NSLOT], [0, KB]],
                offset=c * NSLOT,
            )
            iv = dataclasses.replace(
                idrep_sb[:],
                ap=[[E * KB, 128], [0, NSLOT], [1, KB]],
                offset=0,
            )
            eng.tensor_tensor(dst, iv, gv, op=mybir.AluOpType.mult)
            return diag

        oT_sb = None
        diag_pre = [emit_diag(0, eng=nc.vector), emit_diag(1, eng=nc.vector)]
        for c in range(NCHUNK):
            # --- x chunk DMA ---
            x_sb = xp.tile([128, TPC, I], F16, tag="x")
            nc.sync.dma_start(x_sb[:], xh[:, c, :, :])
            diag = diag_pre[c] if c < 2 else emit_diag(c)

            # --- stage 1 + evacuation, per 128-row tile ---
            yT_all = ytp.tile([128, E, TPC, NL, KB], F16, tag="yT")
            for t in range(TPC):
                for l in range(NL):
                    yT_ps = ps_yt.tile([128, E, KB], F32, tag="yTps")
                    nc.tensor.matmul(
                        yT_ps[:],
                        x_sb[l * KB : (l + 1) * KB, t, :],
                        diag[l * KB : (l + 1) * KB, t, :, :],
                        start=True,
                        stop=True,
                        tile_position=(l * KB, 0),
                    )
                    esrc = yT_ps[:]
                    edst = dataclasses.replace(
                        yT_all[:],
                        ap=[[E * TPC * NL * KB, 128], [TPC * NL * KB, E], [1, KB]],
                        offset=t * NL * KB + l * KB,
                    )
                    if l == 0:
                        nc.vector.tensor_copy(edst, esrc)
                    else:
                        nc.scalar.copy(edst, esrc)

            # --- stage 2: out.T accumulation + bias ---
            oT_ps = ps_ot.tile([U, CHUNK], F32, tag="oTps")
            nc.tensor.matmul(
                oT_ps[:],
                beta_sb[:],
                gT_sb[:, c * CHUNK : (c + 1) * CHUNK],
                start=True,
                stop=False,
            )
            for e in range(E):
                for h in range(2):
                    nc.tensor.matmul(
                        oT_ps[:, h * 256 : (h + 1) * 256],
                        alpha_sb[:, e, :],
                        yT_all[:, e, 2 * h : 2 * h + 2, :, :],
                        start=False,
                        stop=(e == E - 1 and h == 1),
                    )

            # --- out.T evacuation (fp32 -> fp16), alternate DVE/ACT per chunk;
            # output DMA'd in groups of 4,4,4,2,2 (shorter kernel tail) ---
            gstart, glen = (c // OGRP * OGRP, OGRP) if c < 12 else (12 + (c - 12) // 2 * 2, 2)
            if c == gstart:
                oT_sb = op.tile([U, OGRP, CHUNK], F16, tag="oT")
            if c % 2 == 0:
                nc.vector.tensor_copy(oT_sb[:, c - gstart, :], oT_ps[:])
            else:
                nc.scalar.copy(oT_sb[:, c - gstart, :], oT_ps[:])
            if c == gstart + glen - 1:
                nc.sync.dma_start(
                    outT[:, gstart * CHUNK : (gstart + glen) * CHUNK],
                    oT_sb[:, :glen, :],
                )

    nc.compile()
    return nc


_NC_CACHE = None


def _make_idrep():
    idrep = np.zeros((128, E, KB), BF16)
    for p in range(128):
        idrep[p, :, p % KB] = 1.0
    return idrep


def make_in_maps(x, gate_perc, alpha, beta):
    x = np.asarray(x, dtype=np.float32)
    gate_perc = np.asarray(gate_perc, dtype=np.float32)
    alpha_h = np.ascontiguousarray(
        np.asarray(alpha, dtype=np.float32).transpose(1, 0, 2).astype(BF16)
    )
    beta_h = np.asarray(beta, dtype=np.float32).astype(BF16)
    idrep = _make_idrep()
    in_maps = []
    for c in range(NCORES):
        sl = slice(c * BLOC, (c + 1) * BLOC)
        xc = x[sl].astype(BF16)
        gc = gate_perc[sl].astype(BF16)
        # [BLOC, I] -> [128, NCHUNK, TPC, I]: row b = chunk*512 + t*128 + p
        xh = np.ascontiguousarray(
            xc.reshape(NCHUNK, TPC, 128, I).transpose(2, 0, 1, 3)
        )
        gh = np.ascontiguousarray(
            gc.reshape(NCHUNK, TPC, 128, E).transpose(2, 0, 1, 3)
        )
        gT = np.ascontiguousarray(gc.T)
        in_maps.append(
            {
                "xh": xh,
                "gh": gh,
                "gT": gT,
                "alpha": alpha_h,
                "beta": beta_h,
                "idrep": idrep,
            }
        )
    return in_maps


def assemble_output(results):
    # per-core outputs are fp16 [U, BLOC]; concat, cast, transpose on host
    full_T = np.concatenate([results[c]["outT"] for c in range(NCORES)], axis=1)
    return np.ascontiguousarray(full_T.astype(np.float32).T)


def kernel(x, gate_perc, alpha, beta):
    global _NC_CACHE
    if _NC_CACHE is None:
        _NC_CACHE = _build()
    nc = _NC_CACHE

    in_maps = make_in_maps(x, gate_perc, alpha, beta)
    res = run_bass_kernel_spmd(nc, in_maps, list(range(NCORES))).results
    return assemble_output(res)


if __name__ == "__main__":
    rng = np.random.default_rng(0)
    x = rng.standard_normal((B, I)).astype(np.float32)
    g = rng.random((B, E)).astype(np.float32)
    g /= g.sum(-1, keepdims=True)
    al = (rng.standard_normal((E, I, U)) * 0.05).astype(np.float32)
    be = (rng.standard_normal((E, U)) * 0.05).astype(np.float32)
    got = kernel(x, g, al, be)
    ref = np.einsum("bi,eio->beo", x, al, optimize=True)
    ref = np.einsum("beo,be->bo", ref, g) + g @ be
    err = np.abs(got - ref)
    print("max abs err", err.max(), "rel", err.max() / np.abs(ref).max())


# revision 43
# speedup vs baseline: 1.0299x; 1.0299x over previous
"""Trainium2 Bass kernel for nn_DenseExpert (soft-gated mixture of dense experts).

Math:  out[b,u] = sum_e gate[b,e] * (x[b,:] @ alpha[e]) [u] + (gate @ beta)[b,u]

Strategy (pure data parallel over batch, 8 cores). Host pre-casts everything to
bf16 and pre-arranges layouts (x chunked partition-major, gate both batch-major
and pre-transposed, alpha as [i,e,u]) so the device does no casts and no gate
transposes. Per core (8192 rows, 16 chunks of 512):
  1. One-shot DMAs land gate/gateT/alpha/beta/idrep in SBUF; x streams per
     chunk. A few warmup matmuls flip the PE HAM clock-gate to full rate
     while the first DMAs land.
  2. Per chunk, GpSimd builds the block-diagonal gate tensor
     diag[p, (t,e), c] = gate[p,c,(t,e)] * [c == p%64] (one tensor_tensor
     against a 0/1 pattern with a broadcast AP).
  3. Stage 1 (PE): per 128-row tile, 2 row-tiled concurrent matmuls
     (tile_position=(64l,0), K=64, N=512) compute the gate-scaled transpose
     yT[i, (e, c)] = x_block.T @ diag_block into single-bank fp32 PSUM tiles.
  4. DVE (l=0) and ACT (l=1) evacuate the two banks of each tile in parallel
     to bf16 [i, e, b] SBUF layout (1-bank tiles make PSUM slots recycle at
     half-tile granularity, which keeps the PE fed).
  5. Stage 2 (PE): out.T[u, b] = sum_e alpha_e.T @ yT_e accumulated in PSUM;
     the bias rides in the same accumulation group as a leading K=8 matmul
     beta.T @ gateT, and each expert is split into two N=256 halves so the
     first half only waits on the first two tiles' evacuations.
  6. out.T evacuated fp32->bf16 (DVE/ACT alternating per chunk), DMA'd in
     4-chunk groups; host concatenates, casts to fp32, transposes.
"""

import dataclasses
from contextlib import ExitStack

import numpy as np
import ml_dtypes

BF16 = ml_dtypes.bfloat16

import concourse.bacc as bacc
import concourse.tile as tile
import concourse.mybir as mybir
from concourse.bass_utils import run_bass_kernel_spmd

F32 = mybir.dt.float32
F16 = mybir.dt.bfloat16

B, E, I, U = 65536, 8, 128, 128
NCORES = 8
BLOC = B // NCORES        # 8192 batch rows per core
CHUNK = 512               # batch rows per pipeline chunk
NCHUNK = BLOC // CHUNK    # 16
TPC = CHUNK // 128        # 128-row tiles per chunk (4)
KB = 64                   # contraction block for the diag trick
NL = 128 // KB            # row-tile blocks per 128-row tile (4)
OGRP = 4                  # chunks per output DMA group


def _build():
    nc = bacc.Bacc("TRN2", target_bir_lowering=False, debug=False)

    # host-prearranged fp16 inputs
    xh = nc.dram_tensor("xh", [128, NCHUNK, TPC, I], F16, kind="ExternalInput").ap()
    gh = nc.dram_tensor("gh", [128, NCHUNK, TPC, E], F16, kind="ExternalInput").ap()
    gT = nc.dram_tensor("gT", [E, BLOC], F16, kind="ExternalInput").ap()
    alpha = nc.dram_tensor("alpha", [I, E, U], F16, kind="ExternalInput").ap()
    beta = nc.dram_tensor("beta", [E, U], F16, kind="ExternalInput").ap()
    idrep = nc.dram_tensor("idrep", [128, E, KB], F16, kind="ExternalInput").ap()
    # output stays feature-major fp16 on HW; host casts + transposes
    outT = nc.dram_tensor("outT", [U, BLOC], F16, kind="ExternalOutput").ap()

    with tile.TileContext(nc) as tc, ExitStack() as ctx:
        const = ctx.enter_context(tc.tile_pool(name="const", bufs=1))
        xp = ctx.enter_context(tc.tile_pool(name="xp", bufs=6))
        dgp = ctx.enter_context(tc.tile_pool(name="dgp", bufs=3))
        ytp = ctx.enter_context(tc.tile_pool(name="ytp", bufs=3))
        op = ctx.enter_context(tc.tile_pool(name="op", bufs=2))
        ps_yt = ctx.enter_context(tc.tile_pool(name="ps_yt", bufs=6, space="PSUM"))
        ps_ot = ctx.enter_context(tc.tile_pool(name="ps_ot", bufs=2, space="PSUM"))

        # --- one-shot input DMAs (x streamed per chunk below) ---
        idrep_sb = const.tile([128, E, KB], F16, tag="idrep")
        nc.sync.dma_start(idrep_sb[:], idrep)
        gh_sb = const.tile([128, NCHUNK, TPC, E], F16, tag="gh")
        nc.sync.dma_start(gh_sb[:], gh)
        beta_sb = const.tile([E, U], F16, tag="beta")
        nc.sync.dma_start(beta_sb[:], beta)
        alpha_sb = const.tile([I, E, U], F16, tag="alpha")
        nc.sync.dma_start(alpha_sb[:], alpha)
        gT_sb = const.tile([E, BLOC], F16, tag="gT")
        nc.sync.dma_start(gT_sb[:], gT)


        # --- PE warmup: flip HAM to 8/8 while input DMAs land; the same
        # scratch tile is reused for keep-alive matmuls during the ramp so
        # the clock gate never drops while GpSimd catches up on diag builds
        warm_ps = ps_yt.tile([KB, E * KB], F32, tag="yTps")

        def keep_alive(n):
            for _ in range(n):
                nc.tensor.matmul(
                    warm_ps[:],
                    idrep_sb[:, 0, :],
                    dataclasses.replace(
                        idrep_sb[:], ap=[[E * KB, 128], [1, E * KB]], offset=0
                    ),
                    start=True,
                    stop=True,
                )

        keep_alive(6)

        gh_flat = E * TPC * NCHUNK  # per-partition elements of gh_sb
        NSLOT = TPC * E
        DSLOT = 8

        def emit_diag(c, eng=nc.gpsimd):
            # diag[p,(te),cc] = gh[p,c,(te)]*[cc==p%KB], one op per chunk;
            # chunks 0-1 are built on DVE in the prologue (it is idle until
            # the first evacuations), so GpSimd starts directly on diag(2)
            diag = dgp.tile([128, TPC, E, KB], F16, tag="diag")
            dst = dataclasses.replace(
                diag[:],
                ap=[[NSLOT * KB, 128], [KB, NSLOT], [1, KB]],
                offset=0,
            )
            gv = dataclasses.replace(
                gh_sb[:],
                ap=[[gh_flat, 128], [1, NSLOT], [0, KB]],
                offset=c * NSLOT,
            )
            iv = dataclasses.replace(
                idrep_sb[:],
                ap=[[E * KB, 128], [0, NSLOT], [1, KB]],
                offset=0,
            )
            eng.tensor_tensor(dst, iv, gv, op=mybir.AluOpType.mult)
            return diag

        oT_sb = None
        for c in range(NCHUNK):
            # --- x chunk DMA ---
            x_sb = xp.tile([128, TPC, I], F16, tag="x")
            nc.sync.dma_start(x_sb[:], xh[:, c, :, :])
            diag = emit_diag(c)
            if c == 2:
                keep_alive(8)
            elif c == 3:
                keep_alive(4)

            # --- stage 1 + evacuation, per 128-row tile ---
            yT_all = ytp.tile([128, E, TPC, NL, KB], F16, tag="yT")
            for t in range(TPC):
                for l in range(NL):
                    yT_ps = ps_yt.tile([128, E, KB], F32, tag="yTps")
                    nc.tensor.matmul(
                        yT_ps[:],
                        x_sb[l * KB : (l + 1) * KB, t, :],
                        diag[l * KB : (l + 1) * KB, t, :, :],
                        start=True,
                        stop=True,
                        tile_position=(l * KB, 0),
                    )
                    esrc = yT_ps[:]
                    edst = dataclasses.replace(
                        yT_all[:],
                        ap=[[E * TPC * NL * KB, 128], [TPC * NL * KB, E], [1, KB]],
                        offset=t * NL * KB + l * KB,
                    )
                    if l == 0:
                        nc.vector.tensor_copy(edst, esrc)
                    else:
                        nc.scalar.copy(edst, esrc)

            # --- stage 2: out.T accumulation + bias ---
            oT_ps = ps_ot.tile([U, CHUNK], F32, tag="oTps")
            nc.tensor.matmul(
                oT_ps[:],
                beta_sb[:],
                gT_sb[:, c * CHUNK : (c + 1) * CHUNK],
                start=True,
                stop=False,
            )
            for e in range(E):
                for h in range(2):
                    nc.tensor.matmul(
                        oT_ps[:, h * 256 : (h + 1) * 256],
                        alpha_sb[:, e, :],
                        yT_all[:, e, 2 * h : 2 * h + 2, :, :],
                        start=False,
                        stop=(e == E - 1 and h == 1),
                    )

            # --- out.T evacuation (fp32 -> fp16), alternate DVE/ACT per chunk;
            # output DMA'd in groups of 4,4,4,2,2 (shorter kernel tail) ---
            gstart, glen = (c // OGRP * OGRP, OGRP) if c < 12 else (12 + (c - 12) // 2 * 2, 2)
            if c == gstart:
                oT_sb = op.tile([U, OGRP, CHUNK], F16, tag="oT")
            if c % 2 == 0:
                nc.vector.tensor_copy(oT_sb[:, c - gstart, :], oT_ps[:])
            else:
                nc.scalar.copy(oT_sb[:, c - gstart, :], oT_ps[:])
            if c == gstart + glen - 1:
                nc.sync.dma_start(
                    outT[:, gstart * CHUNK : (gstart + glen) * CHUNK],
                    oT_sb[:, :glen, :],
                )

    nc.compile()
    return nc


_NC_CACHE = None


def _make_idrep():
    idrep = np.zeros((128, E, KB), BF16)
    for p in range(128):
        idrep[p, :, p % KB] = 1.0
    return idrep


def make_in_maps(x, gate_perc, alpha, beta):
    x = np.asarray(x, dtype=np.float32)
    gate_perc = np.asarray(gate_perc, dtype=np.float32)
    alpha_h = np.ascontiguousarray(
        np.asarray(alpha, dtype=np.float32).transpose(1, 0, 2).astype(BF16)
    )
    beta_h = np.asarray(beta, dtype=np.float32).astype(BF16)
    idrep = _make_idrep()
    in_maps = []
    for c in range(NCORES):
        sl = slice(c * BLOC, (c + 1) * BLOC)
        xc = x[sl].astype(BF16)
        gc = gate_perc[sl].astype(BF16)
        # [BLOC, I] -> [128, NCHUNK, TPC, I]: row b = chunk*512 + t*128 + p
        xh = np.ascontiguousarray(
            xc.reshape(NCHUNK, TPC, 128, I).transpose(2, 0, 1, 3)
        )
        gh = np.ascontiguousarray(
            gc.reshape(NCHUNK, TPC, 128, E).transpose(2, 0, 1, 3)
        )
        gT = np.ascontiguousarray(gc.T)
        in_maps.append(
            {
                "xh": xh,
                "gh": gh,
                "gT": gT,
                "alpha": alpha_h,
                "beta": beta_h,
                "idrep": idrep,
            }
        )
    return in_maps


def assemble_output(results):
    # per-core outputs are fp16 [U, BLOC]; concat, cast, transpose on host
    full_T = np.concatenate([results[c]["outT"] for c in range(NCORES)], axis=1)
    return np.ascontiguousarray(full_T.astype(np.float32).T)


def kernel(x, gate_perc, alpha, beta):
    global _NC_CACHE
    if _NC_CACHE is None:
        _NC_CACHE = _build()
    nc = _NC_CACHE

    in_maps = make_in_maps(x, gate_perc, alpha, beta)
    res = run_bass_kernel_spmd(nc, in_maps, list(range(NCORES))).results
    return assemble_output(res)


if __name__ == "__main__":
    rng = np.random.default_rng(0)
    x = rng.standard_normal((B, I)).astype(np.float32)
    g = rng.random((B, E)).astype(np.float32)
    g /= g.sum(-1, keepdims=True)
    al = (rng.standard_normal((E, I, U)) * 0.05).astype(np.float32)
    be = (rng.standard_normal((E, U)) * 0.05).astype(np.float32)
    got = kernel(x, g, al, be)
    ref = np.einsum("bi,eio->beo", x, al, optimize=True)
    ref = np.einsum("beo,be->bo", ref, g) + g @ be
    err = np.abs(got - ref)
    print("max abs err", err.max(), "rel", err.max() / np.abs(ref).max())
